# revision 32
# baseline (speedup 1.0000x reference)
"""Trainium2 Bass kernel for the linear state-space scan (nn_ModelBLA).

Reference: un=(u-um)/us; y_n = ys*(C x_n + D un_n)+ym; x_{n+1} = A x_n + B un_n;
returns (y (16384,2,256), X (16384,8,256)) with X[n]=x_n.

Strategy (v2): R sharded over 8 cores (RL=32/core). The linear recurrence is
reformulated as dense matmuls with host-precomputed coefficient matrices (from
the tiny A/B/C/D/mean/std inputs). Blocks of T=16 steps, 16 blocks/group
(256 ts), groups batched x4 (super-groups):
  lvl1: block accumulators c = W1.T @ u, blocks on the FREE axis (one rhs
        layout for everything -> u is read once), written to a DRAM ring;
  lvl2: per group, one K=128 matmul over the sliding 16-block c-window +
        one K=8 anchor matmul; slot layout (b+1)%16 keeps the chain anchor
        at partition 0;
  main: one (40,128) X-matmul + one (40,32) y-matmul per group on a shared
        (40,2048) rhs; X/y tiles assembled 4-groups-wide before storing.
Normalization folds into coefficients; affine constants fold into per-partition
biases on the PSUM->SBUF copies. Outputs land in tile layout; host unpacks.
"""

import numpy as np

N, NU, R, NX, NY = 16384, 2, 256, 8, 2
T = 16
BPG = 16
G = N // (T * BPG)          # 64 groups
B4 = 8                      # groups per batch
SG = G // B4                # 8 batches
RL = R // 8
NCORES = 8

_DT = "float32r"            # matmul operand dtype tag


# ---------------------------------------------------------------- coefficients
def _build_coeffs(A, B_u, C_y, D_yu, um, us, ym, ys):
    A = A.astype(np.float64)
    Bt = B_u.astype(np.float64) / us.astype(np.float64)[None, :]
    C = C_y.astype(np.float64)
    Dt = D_yu.astype(np.float64) / us.astype(np.float64)[None, :]
    um = um.astype(np.float64)
    ys = ys.astype(np.float64)
    ym_ = ym.astype(np.float64)

    Ap = [np.eye(NX)]
    for _ in range(T):
        Ap.append(A @ Ap[-1])
    A16 = Ap[T]
    A16p = [np.eye(NX)]
    for _ in range(BPG + 1):
        A16p.append(A16 @ A16p[-1])
    S = [np.zeros((NX, NX))]
    for t in range(T):
        S.append(S[-1] + Ap[t])
    S16p = [np.zeros((NX, NX))]
    for b in range(BPG + 1):
        S16p.append(S16p[-1] + A16p[b])

    bm = Bt @ um
    cc = -S[T] @ bm

    W1 = np.zeros((32, 8))
    for t in range(T):
        coef = Ap[T - 1 - t] @ Bt
        for iu in range(NU):
            W1[t * 2 + iu, :] = coef[:, iu]

    W2c = np.zeros((128, 128))
    W2s = np.zeros((8, 128))
    W2s0 = np.zeros((8, 128))
    cxb = np.zeros(128)
    cxb0 = np.zeros(128)
    for b in range(BPG):
        m0 = 8 * ((b + 1) % BPG)
        W2s[:, m0:m0 + 8] = A16p[b + 1].T
        W2s0[:, m0:m0 + 8] = A16p[b].T
        cxb[m0:m0 + 8] = S16p[b + 1] @ cc
        cxb0[m0:m0 + 8] = S16p[b] @ cc
        for s in range(b + 1):
            W2c[s * 8:s * 8 + 8, m0:m0 + 8] = A16p[b - s].T

    WX = np.zeros((40, 128))
    cX = np.zeros(128)
    for t in range(T):
        c0 = t * 8
        WX[32:40, c0:c0 + 8] = Ap[t].T
        cX[c0:c0 + 8] = -(S[t] @ bm)
        for tp in range(t):
            coef = Ap[t - 1 - tp] @ Bt
            for iu in range(NU):
                WX[tp * 2 + iu, c0:c0 + 8] = coef[:, iu]

    Wy = np.zeros((40, 32))
    cy = np.zeros(32)
    SyC = np.diag(ys) @ C
    for t in range(T):
        c0 = t * 2
        Wy[32:40, c0:c0 + 2] = (SyC @ Ap[t]).T
        cy[c0:c0 + 2] = ys * (C @ (-(S[t] @ bm)) - Dt @ um) + ym_
        for tp in range(t):
            coef = SyC @ Ap[t - 1 - tp] @ Bt
            for iu in range(NU):
                Wy[tp * 2 + iu, c0:c0 + 2] = coef[:, iu]
        for iu in range(NU):
            Wy[t * 2 + iu, c0:c0 + 2] = ys * Dt[:, iu]

    f32 = lambda x: np.ascontiguousarray(x, dtype=np.float32)
    return dict(W1=f32(W1), W2c=f32(W2c), W2s=f32(W2s), W2s0=f32(W2s0),
                cxb=f32(cxb.reshape(128, 1)), cxb0=f32(cxb0.reshape(128, 1)),
                WX=f32(WX), cX=f32(cX.reshape(128, 1)),
                Wy=f32(Wy), cy=f32(cy.reshape(32, 1)))


# ---------------------------------------------------------------- bass module
_MODULE_CACHE = {}


def _build_module():
    key = _DT
    if key in _MODULE_CACHE:
        return _MODULE_CACHE[key]

    import concourse.bacc as bacc
    import concourse.mybir as mybir
    import concourse.tile as tile

    f32 = mybir.dt.float32
    mdt = getattr(mybir.dt, _DT)
    AF = mybir.ActivationFunctionType

    nc = bacc.Bacc()

    u_d = nc.declare_dram_parameter("u_sh", [N, NU, RL], mdt, isOutput=False)
    x0_d = nc.declare_dram_parameter("x0_sh", [NX, RL], mdt, isOutput=False)
    W1_d = nc.declare_dram_parameter("W1", [32, 8], mdt, isOutput=False)
    W2c_d = nc.declare_dram_parameter("W2c", [128, 128], mdt, isOutput=False)
    W2s_d = nc.declare_dram_parameter("W2s", [8, 128], mdt, isOutput=False)
    W2s0_d = nc.declare_dram_parameter("W2s0", [8, 128], mdt, isOutput=False)
    WX_d = nc.declare_dram_parameter("WX", [40, 128], mdt, isOutput=False)
    Wy_d = nc.declare_dram_parameter("Wy", [40, 32], mdt, isOutput=False)
    cxb_d = nc.declare_dram_parameter("cxb", [128, 1], f32, isOutput=False)
    cxb0_d = nc.declare_dram_parameter("cxb0", [128, 1], f32, isOutput=False)
    cX_d = nc.declare_dram_parameter("cX", [128, 1], f32, isOutput=False)
    cy_d = nc.declare_dram_parameter("cy", [32, 1], f32, isOutput=False)
    zz_d = nc.declare_dram_parameter("zz", [NX, RL], mdt, isOutput=False)
    Xout_d = nc.declare_dram_parameter("Xout", [G, 128, 512], f32, isOutput=True)
    yout_d = nc.declare_dram_parameter("yout", [SG, 32, B4 * 512], f32, isOutput=True)

    # n = (4S+g)*256 + b*16 + t ; free axes everywhere are plain (g, b, r)
    u_ap = u_d.rearrange("(S g b t) u r -> S (t u) g b r", g=B4, b=BPG, t=T)

    with tile.TileContext(nc) as tc:
        with (
            tc.tile_pool(name="consts", bufs=1) as consts,
            tc.tile_pool(name="prhs", bufs=2) as prhs,
            tc.tile_pool(name="pcsb", bufs=2) as pcsb,
            tc.tile_pool(name="pcw", bufs=3) as pcw,
            tc.tile_pool(name="pxb", bufs=2) as pxb,
            tc.tile_pool(name="psbX", bufs=2) as psbX,
            tc.tile_pool(name="psby", bufs=2) as psby,
            tc.tile_pool(name="psc", bufs=2, space="PSUM") as psc,
            tc.tile_pool(name="psxb", bufs=2, space="PSUM") as psxb,
            tc.tile_pool(name="psX", bufs=2, space="PSUM") as psX,
            tc.tile_pool(name="psy", bufs=2, space="PSUM") as psy,
            tc.tile_pool(name="pdc", bufs=1, space="DRAM") as pdc,
            tc.tile_pool(name="pds", bufs=3, space="DRAM") as pds,
        ):
            w1_sb = consts.tile([32, 8], mdt)
            w2c_sb = consts.tile([128, 128], mdt)
            w2s_sb = consts.tile([8, 128], mdt)
            w2s0_sb = consts.tile([8, 128], mdt)
            wx_sb = consts.tile([40, 128], mdt)
            wy_sb = consts.tile([40, 32], mdt)
            cxb_sb = consts.tile([128, 1], f32)
            cxb0_sb = consts.tile([128, 1], f32)
            cX_sb = consts.tile([128, 1], f32)
            cy_sb = consts.tile([32, 1], f32)
            x0_sb = consts.tile([NX, RL], mdt)
            zero_sb = consts.tile([NX, RL], mdt)
            for sb, d in [(w1_sb, W1_d), (w2c_sb, W2c_d), (w2s_sb, W2s_d),
                          (w2s0_sb, W2s0_d), (wx_sb, WX_d), (wy_sb, Wy_d),
                          (cxb_sb, cxb_d), (cxb0_sb, cxb0_d), (cX_sb, cX_d),
                          (cy_sb, cy_d), (x0_sb, x0_d), (zero_sb, zz_d)]:
                nc.sync.dma_start(out=sb[:], in_=d[:])

            # c ring: c_all[j+1] = c_j, c_all[0] = 0  (j = global block index)
            c_all = pdc.tile([G * BPG + 1, NX, RL], mdt)
            nc.sync.dma_start(out=c_all[0], in_=zero_sb[:])

            xb_prev = None
            for S4 in range(SG):
                # ---- u load straight into the shared rhs (one DMA, b order)
                rhs4 = prhs.tile([40, B4 * BPG * RL], mdt)
                nc.sync.dma_start(
                    out=rhs4[0:32, :].rearrange("p (g b r) -> p g b r",
                                                b=BPG, r=RL),
                    in_=u_ap[S4],
                )

                # ---- lvl1: c for the 64 blocks (blocks on free axis)
                c_sb = pcsb.tile([NX, B4 * BPG * RL], mdt)
                for q in range(B4):
                    pc = psc.tile([NX, 512], f32)
                    nc.tensor.matmul(pc[:], w1_sb[:],
                                     rhs4[0:32, q * 512:(q + 1) * 512],
                                     start=True, stop=True)
                    nc.vector.tensor_copy(c_sb[:, q * 512:(q + 1) * 512], pc[:])

                # ---- write c to the DRAM ring (window rows 16g'+b, b order)
                Wv = c_all[16 * B4 * S4 + 1: 16 * B4 * S4 + 16 * B4 + 1]
                nc.gpsimd.dma_start(
                    out=Wv.rearrange("(g b) x r -> x g b r", b=BPG),
                    in_=c_sb.rearrange("x (g b r) -> x g b r", b=BPG, r=RL),
                )

                # ---- sliding c-window for the 4 groups
                cw4 = pcw.tile([128, B4 * RL], mdt)
                nc.sync.dma_start(
                    out=cw4.rearrange("p (g r) -> p g r", r=RL),
                    in_=c_all[16 * B4 * S4: 16 * B4 * S4 + 16 * B4]
                        .rearrange("(g s) x r -> (s x) g r", s=BPG),
                )

                # ---- lvl2: one c-window matmul per 4 groups, then the
                # chained anchor matmuls per group
                xb4 = pxb.tile([128, B4 * RL], mdt)
                for h in range(B4 // 4):
                    pb4 = psxb.tile([128, 4 * RL], f32)
                    nc.tensor.matmul(pb4[:], w2c_sb[:],
                                     cw4[:, h * 4 * RL:(h + 1) * 4 * RL],
                                     start=True, stop=False)
                    for gq in range(4):
                        gp = h * 4 + gq
                        g = S4 * B4 + gp
                        if g == 0:
                            nc.tensor.matmul(pb4[:, 0:RL], w2s0_sb[:],
                                             x0_sb[:],
                                             start=False, stop=False,
                                             skip_group_check=True)
                            bias = cxb0_sb
                        else:
                            anchor = (xb_prev if gp == 0 else xb4)
                            off = ((B4 - 1) * RL if gp == 0
                                   else (gp - 1) * RL)
                            nc.tensor.matmul(pb4[:, gq * RL:(gq + 1) * RL],
                                             w2s_sb[:],
                                             anchor[0:8, off:off + RL],
                                             start=False, stop=(gq == 3),
                                             skip_group_check=True)
                            bias = cxb_sb
                        nc.scalar.activation(xb4[:, gp * RL:(gp + 1) * RL],
                                             pb4[:, gq * RL:(gq + 1) * RL],
                                             AF.Identity, bias=bias[:])
                xb_prev = xb4

                # ---- bounce xb to DRAM (b-ordered), read back as rhs rows
                # xb4 partitions are slot-ordered (s = (b+1)%16, anchor at 0)
                scr = pds.tile([B4, BPG, NX, RL], mdt)      # [g', b, jx, r]
                nc.gpsimd.dma_start(
                    out=scr[:, 0:BPG - 1].rearrange("g b j r -> b j g r"),
                    in_=xb4[8:128, :],
                )
                nc.gpsimd.dma_start(
                    out=scr[:, BPG - 1].rearrange("g j r -> j g r"),
                    in_=xb4[0:8, :],
                )
                nc.sync.dma_start(
                    out=rhs4[32:40, :].rearrange("j (g b r) -> j g b r",
                                                 b=BPG, r=RL),
                    in_=scr.rearrange("g b j r -> j g b r"),
                )

                # ---- main matmuls + biased copies + stores
                sX4 = psbX.tile([128, B4 * 512], f32)
                sy4 = psby.tile([32, B4 * 512], f32)
                for gp in range(B4):
                    pX = psX.tile([128, 512], f32)
                    nc.tensor.matmul(pX[:], wx_sb[:],
                                     rhs4[:, gp * 512:(gp + 1) * 512],
                                     start=True, stop=True)
                    nc.vector.tensor_scalar_add(
                        sX4[:, gp * 512:(gp + 1) * 512], pX[:], cX_sb[:])
                    py = psy.tile([32, 512], f32)
                    nc.tensor.matmul(py[:], wy_sb[:],
                                     rhs4[:, gp * 512:(gp + 1) * 512],
                                     start=True, stop=True)
                    nc.scalar.activation(sy4[:, gp * 512:(gp + 1) * 512],
                                         py[:], AF.Identity, bias=cy_sb[:])
                nc.scalar.dma_start(
                    out=Xout_d[B4 * S4: B4 * S4 + B4].rearrange("g p f -> p g f"),
                    in_=sX4[:].rearrange("p (g f) -> p g f", f=512),
                )
                nc.scalar.dma_start(out=yout_d[S4], in_=sy4[:])

    nc.finalize()
    _MODULE_CACHE[key] = nc
    return nc


# ---------------------------------------------------------------- host wrapper
def _make_in_maps(u, x0, A, B_u, C_y, D_yu, u_mean, u_std, y_mean, y_std):
    cf = _build_coeffs(A, B_u, C_y, D_yu, u_mean, u_std, y_mean, y_std)
    in_maps = []
    for c in range(NCORES):
        r0 = c * RL
        in_maps.append({
            "u_sh": np.ascontiguousarray(u[:, :, r0:r0 + RL], dtype=np.float32),
            "x0_sh": np.ascontiguousarray(x0[:, r0:r0 + RL], dtype=np.float32),
            "zz": np.zeros((NX, RL), np.float32),
            **{k: cf[k] for k in ("W1", "W2c", "W2s", "W2s0", "WX", "Wy",
                                  "cxb", "cxb0", "cX", "cy")},
        })
    return in_maps


def _unpack(results):
    ys, Xs = [], []
    for res in results:
        Xout = np.asarray(res["Xout"])
        yout = np.asarray(res["yout"])
        Xs.append(Xout.reshape(G, T, 8, BPG, RL)
                  .transpose(0, 3, 1, 2, 4).reshape(N, 8, RL))
        ys.append(yout.reshape(SG, T, 2, B4, BPG, RL)
                  .transpose(0, 3, 4, 1, 2, 5).reshape(N, 2, RL))
    return np.concatenate(ys, axis=2), np.concatenate(Xs, axis=2)


def run(inputs, trace=False):
    """Build+run on 8 cores. Returns ((y, X), exec_time_ns_or_None)."""
    from concourse.bass_utils import run_bass_kernel_spmd

    nc = _build_module()
    in_maps = _make_in_maps(**inputs)
    res = run_bass_kernel_spmd(nc, in_maps, list(range(NCORES)), trace=trace)
    y, X = _unpack(res.results)
    return (y, X), getattr(res, "exec_time_ns", None)


def kernel(u, x0, A, B_u, C_y, D_yu, u_mean, u_std, y_mean, y_std):
    (y, X), _ = run(dict(u=u, x0=x0, A=A, B_u=B_u, C_y=C_y, D_yu=D_yu,
                         u_mean=u_mean, u_std=u_std, y_mean=y_mean,
                         y_std=y_std))
    return y, X


# revision 34
# speedup vs baseline: 1.3036x; 1.3036x over previous
"""Trainium2 Bass kernel for the linear state-space scan (nn_ModelBLA).

Reference: un=(u-um)/us; y_n = ys*(C x_n + D un_n)+ym; x_{n+1} = A x_n + B un_n;
returns (y (16384,2,256), X (16384,8,256)) with X[n]=x_n.

Strategy (v2): R sharded over 8 cores (RL=32/core). The linear recurrence is
reformulated as dense matmuls with host-precomputed coefficient matrices (from
the tiny A/B/C/D/mean/std inputs). Blocks of T=16 steps, 16 blocks/group
(256 ts), groups batched x4 (super-groups):
  lvl1: block accumulators c = W1.T @ u, blocks on the FREE axis (one rhs
        layout for everything -> u is read once), written to a DRAM ring;
  lvl2: per group, one K=128 matmul over the sliding 16-block c-window +
        one K=8 anchor matmul; slot layout (b+1)%16 keeps the chain anchor
        at partition 0;
  main: one (40,128) X-matmul + one (40,32) y-matmul per group on a shared
        (40,2048) rhs; y matmuls col-tiled 4-to-a-PSUM-bank.
Normalization folds into coefficients; affine constants fold into per-partition
biases on the PSUM->SBUF copies. Outputs land in tile layout; host unpacks.
"""

import numpy as np

N, NU, R, NX, NY = 16384, 2, 256, 8, 2
T = 16
BPG = 16
G = N // (T * BPG)          # 64 groups
SG = G // 4                 # 16 super-groups
RL = R // 8
NCORES = 8

_DT = "float32r"            # matmul operand dtype tag


# ---------------------------------------------------------------- coefficients
def _build_coeffs(A, B_u, C_y, D_yu, um, us, ym, ys):
    A = A.astype(np.float64)
    Bt = B_u.astype(np.float64) / us.astype(np.float64)[None, :]
    C = C_y.astype(np.float64)
    Dt = D_yu.astype(np.float64) / us.astype(np.float64)[None, :]
    um = um.astype(np.float64)
    ys = ys.astype(np.float64)
    ym_ = ym.astype(np.float64)

    Ap = [np.eye(NX)]
    for _ in range(T):
        Ap.append(A @ Ap[-1])
    A16 = Ap[T]
    A16p = [np.eye(NX)]
    for _ in range(BPG + 1):
        A16p.append(A16 @ A16p[-1])
    S = [np.zeros((NX, NX))]
    for t in range(T):
        S.append(S[-1] + Ap[t])
    S16p = [np.zeros((NX, NX))]
    for b in range(BPG + 1):
        S16p.append(S16p[-1] + A16p[b])

    bm = Bt @ um
    cc = -S[T] @ bm

    W1 = np.zeros((32, 8))
    for t in range(T):
        coef = Ap[T - 1 - t] @ Bt
        for iu in range(NU):
            W1[t * 2 + iu, :] = coef[:, iu]

    W2c = np.zeros((128, 128))
    W2s = np.zeros((8, 128))
    W2s0 = np.zeros((8, 128))
    cxb = np.zeros(128)
    cxb0 = np.zeros(128)
    for b in range(BPG):
        m0 = 8 * ((b + 1) % BPG)
        W2s[:, m0:m0 + 8] = A16p[b + 1].T
        W2s0[:, m0:m0 + 8] = A16p[b].T
        cxb[m0:m0 + 8] = S16p[b + 1] @ cc
        cxb0[m0:m0 + 8] = S16p[b] @ cc
        for s in range(b + 1):
            W2c[s * 8:s * 8 + 8, m0:m0 + 8] = A16p[b - s].T

    WX = np.zeros((40, 128))
    cX = np.zeros(128)
    for t in range(T):
        c0 = t * 8
        WX[32:40, c0:c0 + 8] = Ap[t].T
        cX[c0:c0 + 8] = -(S[t] @ bm)
        for tp in range(t):
            coef = Ap[t - 1 - tp] @ Bt
            for iu in range(NU):
                WX[tp * 2 + iu, c0:c0 + 8] = coef[:, iu]

    Wy = np.zeros((40, 32))
    cy = np.zeros(32)
    SyC = np.diag(ys) @ C
    for t in range(T):
        c0 = t * 2
        Wy[32:40, c0:c0 + 2] = (SyC @ Ap[t]).T
        cy[c0:c0 + 2] = ys * (C @ (-(S[t] @ bm)) - Dt @ um) + ym_
        for tp in range(t):
            coef = SyC @ Ap[t - 1 - tp] @ Bt
            for iu in range(NU):
                Wy[tp * 2 + iu, c0:c0 + 2] = coef[:, iu]
        for iu in range(NU):
            Wy[t * 2 + iu, c0:c0 + 2] = ys * Dt[:, iu]

    f32 = lambda x: np.ascontiguousarray(x, dtype=np.float32)
    return dict(W1=f32(W1), W2c=f32(W2c), W2s=f32(W2s), W2s0=f32(W2s0),
                cxb=f32(cxb.reshape(128, 1)), cxb0=f32(cxb0.reshape(128, 1)),
                WX=f32(WX), cX=f32(cX.reshape(128, 1)),
                Wy=f32(Wy), cy=f32(cy.reshape(32, 1)))


# ---------------------------------------------------------------- bass module
_MODULE_CACHE = {}


def _build_module():
    key = _DT
    if key in _MODULE_CACHE:
        return _MODULE_CACHE[key]

    import concourse.bacc as bacc
    import concourse.mybir as mybir
    import concourse.tile as tile

    f32 = mybir.dt.float32
    mdt = getattr(mybir.dt, _DT)
    AF = mybir.ActivationFunctionType

    nc = bacc.Bacc()

    u_d = nc.declare_dram_parameter("u_sh", [N, NU, RL], mdt, isOutput=False)
    x0_d = nc.declare_dram_parameter("x0_sh", [NX, RL], mdt, isOutput=False)
    W1_d = nc.declare_dram_parameter("W1", [32, 8], mdt, isOutput=False)
    W2c_d = nc.declare_dram_parameter("W2c", [128, 128], mdt, isOutput=False)
    W2s_d = nc.declare_dram_parameter("W2s", [8, 128], mdt, isOutput=False)
    W2s0_d = nc.declare_dram_parameter("W2s0", [8, 128], mdt, isOutput=False)
    WX_d = nc.declare_dram_parameter("WX", [40, 128], mdt, isOutput=False)
    Wy_d = nc.declare_dram_parameter("Wy", [40, 32], mdt, isOutput=False)
    cxb_d = nc.declare_dram_parameter("cxb", [128, 1], f32, isOutput=False)
    cxb0_d = nc.declare_dram_parameter("cxb0", [128, 1], f32, isOutput=False)
    cX_d = nc.declare_dram_parameter("cX", [128, 1], f32, isOutput=False)
    cy_d = nc.declare_dram_parameter("cy", [32, 1], f32, isOutput=False)
    zz_d = nc.declare_dram_parameter("zz", [NX, RL], mdt, isOutput=False)
    Xout_d = nc.declare_dram_parameter("Xout", [G, 128, 512], f32, isOutput=True)
    yout_d = nc.declare_dram_parameter("yout", [SG, 32, 2048], f32, isOutput=True)

    # n = (4S+g)*256 + b*16 + t ; free axes everywhere are plain (g, b, r)
    u_ap = u_d.rearrange("(S g b t) u r -> S (t u) g b r", g=4, b=BPG, t=T)

    with tile.TileContext(nc) as tc:
        with (
            tc.tile_pool(name="consts", bufs=1) as consts,
            tc.tile_pool(name="prhs", bufs=4) as prhs,
            tc.tile_pool(name="pcsb", bufs=4) as pcsb,
            tc.tile_pool(name="pcw", bufs=4) as pcw,
            tc.tile_pool(name="pxb", bufs=3) as pxb,
            tc.tile_pool(name="psbX", bufs=4) as psbX,
            tc.tile_pool(name="psby", bufs=4) as psby,
            tc.tile_pool(name="psc", bufs=2, space="PSUM") as psc,
            tc.tile_pool(name="psxb", bufs=2, space="PSUM") as psxb,
            tc.tile_pool(name="psX", bufs=2, space="PSUM") as psX,
            tc.tile_pool(name="psy", bufs=2, space="PSUM") as psy,
            tc.tile_pool(name="pdc", bufs=1, space="DRAM") as pdc,
            tc.tile_pool(name="pds", bufs=4, space="DRAM") as pds,
        ):
            w1_sb = consts.tile([32, 8], mdt)
            w2c_sb = consts.tile([128, 128], mdt)
            w2s_sb = consts.tile([8, 128], mdt)
            w2s0_sb = consts.tile([8, 128], mdt)
            wx_sb = consts.tile([40, 128], mdt)
            wy_sb = consts.tile([40, 32], mdt)
            cxb_sb = consts.tile([128, 1], f32)
            cxb0_sb = consts.tile([128, 1], f32)
            cX_sb = consts.tile([128, 1], f32)
            cy_sb = consts.tile([32, 1], f32)
            x0_sb = consts.tile([NX, RL], mdt)
            zero_sb = consts.tile([NX, RL], mdt)
            for sb, d in [(w1_sb, W1_d), (w2c_sb, W2c_d), (w2s_sb, W2s_d),
                          (w2s0_sb, W2s0_d), (wx_sb, WX_d), (wy_sb, Wy_d),
                          (cxb_sb, cxb_d), (cxb0_sb, cxb0_d), (cX_sb, cX_d),
                          (cy_sb, cy_d), (x0_sb, x0_d), (zero_sb, zz_d)]:
                nc.sync.dma_start(out=sb[:], in_=d[:])

            # c ring: c_all[j+1] = c_j, c_all[0] = 0  (j = global block index)
            c_all = pdc.tile([G * BPG + 1, NX, RL], mdt)
            nc.sync.dma_start(out=c_all[0], in_=zero_sb[:])

            xb_prev = None
            for S4 in range(SG):
                # ---- u load straight into the shared rhs (one DMA, b order)
                rhs4 = prhs.tile([40, 4 * BPG * RL], mdt)
                nc.sync.dma_start(
                    out=rhs4[0:32, :].rearrange("p (g b r) -> p g b r",
                                                b=BPG, r=RL),
                    in_=u_ap[S4],
                )

                # ---- lvl1: c for the 64 blocks (blocks on free axis)
                c_sb = pcsb.tile([NX, 4 * BPG * RL], mdt)
                for q in range(4):
                    pc = psc.tile([NX, 512], f32)
                    nc.tensor.matmul(pc[:], w1_sb[:],
                                     rhs4[0:32, q * 512:(q + 1) * 512],
                                     start=True, stop=True)
                    nc.vector.tensor_copy(c_sb[:, q * 512:(q + 1) * 512], pc[:])

                # ---- write c to the DRAM ring (window rows 16g'+b, b order)
                Wv = c_all[64 * S4 + 1: 64 * S4 + 65]
                nc.gpsimd.dma_start(
                    out=Wv.rearrange("(g b) x r -> x g b r", b=BPG),
                    in_=c_sb.rearrange("x (g b r) -> x g b r", b=BPG, r=RL),
                )

                # ---- sliding c-window for the 4 groups
                cw4 = pcw.tile([128, 4 * RL], mdt)
                nc.sync.dma_start(
                    out=cw4.rearrange("p (g r) -> p g r", r=RL),
                    in_=c_all[64 * S4: 64 * S4 + 64]
                        .rearrange("(g s) x r -> (s x) g r", s=BPG),
                )

                # ---- lvl2: one c-window matmul for all 4 groups, then the
                # chained anchor matmuls per group
                xb4 = pxb.tile([128, 4 * RL], mdt)
                pb4 = psxb.tile([128, 4 * RL], f32)
                nc.tensor.matmul(pb4[:], w2c_sb[:], cw4[:],
                                 start=True, stop=False)
                for gp in range(4):
                    g = S4 * 4 + gp
                    if g == 0:
                        nc.tensor.matmul(pb4[:, 0:RL], w2s0_sb[:], x0_sb[:],
                                         start=False, stop=False,
                                         skip_group_check=True)
                        bias = cxb0_sb
                    else:
                        anchor = (xb_prev if gp == 0 else xb4)
                        off = 96 if gp == 0 else (gp - 1) * RL
                        nc.tensor.matmul(pb4[:, gp * RL:(gp + 1) * RL],
                                         w2s_sb[:], anchor[0:8, off:off + RL],
                                         start=False, stop=(gp == 3),
                                         skip_group_check=True)
                        bias = cxb_sb
                    nc.scalar.activation(xb4[:, gp * RL:(gp + 1) * RL],
                                         pb4[:, gp * RL:(gp + 1) * RL],
                                         AF.Identity, bias=bias[:])
                xb_prev = xb4

                # ---- bounce xb to DRAM (b-ordered), read back as rhs rows
                # xb4 partitions are slot-ordered (s = (b+1)%16, anchor at 0)
                scr = pds.tile([4, BPG, NX, RL], mdt)      # [g', b, jx, r]
                nc.gpsimd.dma_start(
                    out=scr[:, 0:BPG - 1].rearrange("g b j r -> b j g r"),
                    in_=xb4[8:128, :],
                )
                nc.gpsimd.dma_start(
                    out=scr[:, BPG - 1].rearrange("g j r -> j g r"),
                    in_=xb4[0:8, :],
                )
                nc.sync.dma_start(
                    out=rhs4[32:40, :].rearrange("j (g b r) -> j g b r",
                                                 b=BPG, r=RL),
                    in_=scr.rearrange("g b j r -> j g b r"),
                )

                # ---- main matmuls + biased copies + stores
                sX4 = psbX.tile([128, 4 * 512], f32)
                sy4 = psby.tile([32, 4 * 512], f32)
                for gp in range(4):
                    pX = psX.tile([128, 512], f32)
                    nc.tensor.matmul(pX[:], wx_sb[:],
                                     rhs4[:, gp * 512:(gp + 1) * 512],
                                     start=True, stop=True)
                    nc.vector.tensor_scalar_add(
                        sX4[:, gp * 512:(gp + 1) * 512], pX[:], cX_sb[:])
                    py = psy.tile([32, 512], f32)
                    nc.tensor.matmul(py[:], wy_sb[:],
                                     rhs4[:, gp * 512:(gp + 1) * 512],
                                     start=True, stop=True)
                    nc.scalar.activation(sy4[:, gp * 512:(gp + 1) * 512],
                                         py[:], AF.Identity, bias=cy_sb[:])
                nc.scalar.dma_start(
                    out=Xout_d[4 * S4: 4 * S4 + 4].rearrange("g p f -> p g f"),
                    in_=sX4[:].rearrange("p (g f) -> p g f", f=512),
                )
                nc.scalar.dma_start(out=yout_d[S4], in_=sy4[:])

    nc.finalize()
    _MODULE_CACHE[key] = nc
    return nc


# ---------------------------------------------------------------- host wrapper
def _make_in_maps(u, x0, A, B_u, C_y, D_yu, u_mean, u_std, y_mean, y_std):
    cf = _build_coeffs(A, B_u, C_y, D_yu, u_mean, u_std, y_mean, y_std)
    in_maps = []
    for c in range(NCORES):
        r0 = c * RL
        in_maps.append({
            "u_sh": np.ascontiguousarray(u[:, :, r0:r0 + RL], dtype=np.float32),
            "x0_sh": np.ascontiguousarray(x0[:, r0:r0 + RL], dtype=np.float32),
            "zz": np.zeros((NX, RL), np.float32),
            **{k: cf[k] for k in ("W1", "W2c", "W2s", "W2s0", "WX", "Wy",
                                  "cxb", "cxb0", "cX", "cy")},
        })
    return in_maps


def _unpack(results):
    ys, Xs = [], []
    for res in results:
        Xout = np.asarray(res["Xout"])
        yout = np.asarray(res["yout"])
        Xs.append(Xout.reshape(G, T, 8, BPG, RL)
                  .transpose(0, 3, 1, 2, 4).reshape(N, 8, RL))
        ys.append(yout.reshape(SG, T, 2, 4, BPG, RL)
                  .transpose(0, 3, 4, 1, 2, 5).reshape(N, 2, RL))
    return np.concatenate(ys, axis=2), np.concatenate(Xs, axis=2)


def run(inputs, trace=False):
    """Build+run on 8 cores. Returns ((y, X), exec_time_ns_or_None)."""
    from concourse.bass_utils import run_bass_kernel_spmd

    nc = _build_module()
    in_maps = _make_in_maps(**inputs)
    res = run_bass_kernel_spmd(nc, in_maps, list(range(NCORES)), trace=trace)
    y, X = _unpack(res.results)
    return (y, X), getattr(res, "exec_time_ns", None)


def kernel(u, x0, A, B_u, C_y, D_yu, u_mean, u_std, y_mean, y_std):
    (y, X), _ = run(dict(u=u, x0=x0, A=A, B_u=B_u, C_y=C_y, D_yu=D_yu,
                         u_mean=u_mean, u_std=u_std, y_mean=y_mean,
                         y_std=y_std))
    return y, X
